# revision 1
# baseline (speedup 1.0000x reference)
"""DeepSeek-V2-style MoE kernel for 8 Trainium2 NeuronCores.

Sharding strategy (expert-parallel + shared-expert channel-parallel):
  - Core n runs routed expert n on ALL tokens (dense dispatch, matching the
    reference), weighted by that expert's per-token combine weight.
  - The always-on shared expert's intermediate dim FS=2816 is sharded 8-ways
    (352 channels/core, padded to 384); the down-projection contracts over
    the local channels only, so each core's shared output is a partial sum.
  - Router (gate matmul + softmax + group-limited top-2) is computed on
    every core in fp32 on the tensor engine; each core extracts its own
    expert's combine-weight column via a one-hot selector input.
  - Each core emits a full [T, H] fp32 partial; the host sums the 8 partials.

Heavy matmuls run in bf16 (fp32 PSUM accumulation). Weight-stationary
ko-outer loops amortize LDWEIGHTS over 4 matmuls; routing is batched into
its own phase so the scalar engine's activation table is loaded once per
function (table swaps cost ~1.3us each).
"""

from contextlib import ExitStack

import numpy as np
import ml_dtypes

import concourse.bass as bass
import concourse.tile as tile
from concourse import bacc, mybir
from concourse.bass_utils import run_bass_kernel_spmd

BF16 = ml_dtypes.bfloat16
F32 = np.float32

P = 128
B, S, H, F, FS, NEXP = 2, 1024, 2048, 1408, 2816, 8
T = B * S                      # 2048 tokens
FSL = FS // NEXP               # 352 shared channels per core
FSLP = 384                     # padded to a multiple of 128
KO = H // P                    # 16 contraction chunks over H
TB = T // P                    # 16 token blocks of 128
TCH = T // 512                 # 4 token chunks of 512
HCH = H // 512                 # 4 output chunks of 512
FBN = F // P                   # 11 expert f-blocks
SBN = FSLP // P                # 3 shared fs-blocks

_X = mybir.AxisListType.X
_ALU = mybir.AluOpType
_ACTF = mybir.ActivationFunctionType
_F32 = mybir.dt.float32
_BF16 = mybir.dt.bfloat16

_CACHED_NC = None


def _build_body(ctx, tc, repeat=1, skip_routing=False, skip_dphase=False,
                skip_mphase=False):
    nc = tc.nc
    hbf_d = nc.dram_tensor("hbf", [P, KO, T], _BF16, kind="ExternalInput").ap()
    hf_d = nc.dram_tensor("hf", [TB, P, KO, P], _F32, kind="ExternalInput").ap()
    gw8_d = nc.dram_tensor("gw8", [P, KO, NEXP], _F32, kind="ExternalInput").ap()
    esel_d = nc.dram_tensor("esel", [P, NEXP], _F32, kind="ExternalInput").ap()
    gwl_d = nc.dram_tensor("gwl", [FBN, P, KO, P], _BF16, kind="ExternalInput").ap()
    uwl_d = nc.dram_tensor("uwl", [FBN, P, KO, P], _BF16, kind="ExternalInput").ap()
    sgl_d = nc.dram_tensor("sgl", [SBN, P, KO, P], _BF16, kind="ExternalInput").ap()
    sul_d = nc.dram_tensor("sul", [SBN, P, KO, P], _BF16, kind="ExternalInput").ap()
    dwl_d = nc.dram_tensor("dwl", [HCH, P, FBN, 512], _BF16, kind="ExternalInput").ap()
    sdl_d = nc.dram_tensor("sdl", [HCH, P, SBN, 512], _BF16, kind="ExternalInput").ap()
    out_d = nc.dram_tensor("out", [T, H], _F32, kind="ExternalOutput").ap()

    consts = ctx.enter_context(tc.tile_pool(name="consts", bufs=1))
    hbf_pool = ctx.enter_context(tc.tile_pool(name="hbfp", bufs=1))
    a_pool = ctx.enter_context(tc.tile_pool(name="apool", bufs=1))
    wpool = ctx.enter_context(tc.tile_pool(name="wpool", bufs=2))
    hfpool = ctx.enter_context(tc.tile_pool(name="hfpool", bufs=2))
    rpool = ctx.enter_context(tc.tile_pool(name="rpool", bufs=2))
    sgpool = ctx.enter_context(tc.tile_pool(name="sgpool", bufs=5))
    dpool = ctx.enter_context(tc.tile_pool(name="dpool", bufs=2))
    opool = ctx.enter_context(tc.tile_pool(name="opool", bufs=3))
    mmp = ctx.enter_context(tc.tile_pool(name="mmp", bufs=1, space="PSUM"))

    gw8_sb = consts.tile([P, KO, NEXP], _F32)
    nc.sync.dma_start(gw8_sb[:], gw8_d[:])
    esel_sb = consts.tile([P, NEXP], _F32)
    nc.sync.dma_start(esel_sb[:], esel_d[:])
    w_all = consts.tile([P, TB], _F32)
    if skip_routing:
        nc.vector.memset(w_all[:], 1.0)

    hbf_sb = hbf_pool.tile([P, KO, T], _BF16)
    nc.sync.dma_start(hbf_sb[:], hbf_d[:])

    aT = a_pool.tile([P, FBN, T], _BF16)
    ash = a_pool.tile([P, SBN, T], _BF16)

    def ffn_unit(wg_src, wu_src, dst, dst_blk):
        """Gate/up matmuls + silu*up for one 128-wide block of the
        intermediate dim. ko-outer: each LDWEIGHTS feeds 4 N=512 matmuls."""
        wg_t = wpool.tile([P, KO, P], _BF16, tag="wg", name="wg_t")
        nc.sync.dma_start(wg_t[:], wg_src)
        wu_t = wpool.tile([P, KO, P], _BF16, tag="wu", name="wu_t")
        nc.sync.dma_start(wu_t[:], wu_src)
        pgs = [mmp.tile([P, 512], _F32, tag="pg", bufs=4, name=f"pg{t}")
               for t in range(TCH)]
        for ko in range(KO):
            for t in range(TCH):
                nc.tensor.matmul(
                    pgs[t][:], wg_t[:, ko, :],
                    hbf_sb[:, ko, t * 512:(t + 1) * 512],
                    start=(ko == 0), stop=(ko == KO - 1),
                )
        pus = [mmp.tile([P, 512], _F32, tag="pu", bufs=3, name=f"pu{t}")
               for t in range(TCH)]
        for ko in range(KO):
            for t in range(TCH):
                nc.tensor.matmul(
                    pus[t][:], wu_t[:, ko, :],
                    hbf_sb[:, ko, t * 512:(t + 1) * 512],
                    start=(ko == 0), stop=(ko == KO - 1),
                )
        for t in range(TCH):
            ts_ = slice(t * 512, (t + 1) * 512)
            sg = sgpool.tile([P, 512], _F32, tag="sg", name="sg")
            nc.scalar.activation(sg[:], pgs[t][:], _ACTF.Sigmoid)
            nc.vector.tensor_tensor(sg[:], sg[:], pgs[t][:], _ALU.mult)
            nc.vector.tensor_tensor(dst[:, dst_blk, ts_], sg[:], pus[t][:],
                                    _ALU.mult)

    def routing_block(j):
        """Router for token block j: fp32 logits -> softmax -> group-limited
        top-2 -> this core's combine-weight column w_all[:, j]."""
        hf_t = hfpool.tile([P, KO, P], _F32, tag="hf", name="hf_t")
        nc.sync.dma_start(hf_t[:], hf_d[j])
        pl = mmp.tile([P, NEXP], _F32, tag="pu", bufs=3, name="pl")
        for ko in range(KO):
            nc.tensor.matmul(
                pl[:], hf_t[:, ko, :], gw8_sb[:, ko, :],
                start=(ko == 0), stop=(ko == KO - 1),
            )
        negmx = rpool.tile([P, 1], _F32, tag="negmx", name="negmx")
        nc.vector.tensor_reduce(negmx[:], pl[:], _X, _ALU.max, negate=True)
        ssum = rpool.tile([P, 1], _F32, tag="ssum", name="ssum")
        sc = rpool.tile([P, NEXP], _F32, tag="sc", name="sc")
        nc.scalar.activation(
            sc[:], pl[:], _ACTF.Exp, bias=negmx[:, 0:1], scale=1.0,
            accum_out=ssum[:, 0:1],
        )
        rec = rpool.tile([P, 1], _F32, tag="rec", name="rec")
        nc.vector.reciprocal(rec[:], ssum[:])
        sc2 = rpool.tile([P, NEXP], _F32, tag="sc2", name="sc2")
        nc.vector.tensor_scalar_mul(sc2[:], sc[:], rec[:, 0:1])
        # group scores: max over pairs of adjacent experts -> [P, 4]
        g = rpool.tile([P, 4], _F32, tag="g", name="g")
        nc.vector.tensor_reduce(
            g[:], sc2.rearrange("p (g e) -> p g e", e=2), _X, _ALU.max
        )
        m1g = rpool.tile([P, 1], _F32, tag="m1g", name="m1g")
        nc.vector.tensor_reduce(m1g[:], g[:], _X, _ALU.max)
        is1 = rpool.tile([P, 4], _F32, tag="is1", name="is1")
        nc.vector.tensor_scalar(is1[:], g[:], m1g[:, 0:1], None, _ALU.is_ge)
        gm = rpool.tile([P, 4], _F32, tag="gm", name="gm")
        nc.vector.scalar_tensor_tensor(
            gm[:], is1[:], -1e30, g[:], _ALU.mult, _ALU.add
        )
        m2g = rpool.tile([P, 1], _F32, tag="m2g", name="m2g")
        nc.vector.tensor_reduce(m2g[:], gm[:], _X, _ALU.max)
        gmask = rpool.tile([P, 4], _F32, tag="gmask", name="gmask")
        nc.vector.tensor_scalar(gmask[:], g[:], m2g[:, 0:1], None, _ALU.is_ge)
        smask = rpool.tile([P, NEXP], _F32, tag="smask", name="smask")
        sm_v = smask.rearrange("p (g e) -> p g e", e=2)
        nc.vector.tensor_copy(sm_v[:, :, 0], gmask[:])
        nc.vector.tensor_copy(sm_v[:, :, 1], gmask[:])
        msk = rpool.tile([P, NEXP], _F32, tag="msk", name="msk")
        nc.vector.tensor_tensor(msk[:], sc2[:], smask[:], _ALU.mult)
        m1e = rpool.tile([P, 1], _F32, tag="m1e", name="m1e")
        nc.vector.tensor_reduce(m1e[:], msk[:], _X, _ALU.max)
        is1e = rpool.tile([P, NEXP], _F32, tag="is1e", name="is1e")
        nc.vector.tensor_scalar(is1e[:], msk[:], m1e[:, 0:1], None, _ALU.is_ge)
        me = rpool.tile([P, NEXP], _F32, tag="me", name="me")
        nc.vector.scalar_tensor_tensor(
            me[:], is1e[:], -1e30, msk[:], _ALU.mult, _ALU.add
        )
        m2e = rpool.tile([P, 1], _F32, tag="m2e", name="m2e")
        nc.vector.tensor_reduce(m2e[:], me[:], _X, _ALU.max)
        wsel = rpool.tile([P, NEXP], _F32, tag="wsel", name="wsel")
        nc.vector.tensor_scalar(wsel[:], msk[:], m2e[:, 0:1], None, _ALU.is_ge)
        wall = rpool.tile([P, NEXP], _F32, tag="wall", name="wall")
        nc.vector.tensor_tensor(wall[:], msk[:], wsel[:], _ALU.mult)
        tmp8 = rpool.tile([P, NEXP], _F32, tag="tmp8", name="tmp8")
        nc.vector.tensor_tensor(tmp8[:], wall[:], esel_sb[:], _ALU.mult)
        nc.vector.tensor_reduce(w_all[:, j : j + 1], tmp8[:], _X, _ALU.add)

    for _rep in range(repeat):
        # ---- M phase: expert + shared gate/up projections ----
        if not skip_mphase:
            for fb in range(FBN):
                ffn_unit(gwl_d[fb], uwl_d[fb], aT, fb)
            for sb in range(SBN):
                ffn_unit(sgl_d[sb], sul_d[sb], ash, sb)
        # ---- R phase: routing (batched: one Exp table load) ----
        if not skip_routing:
            for j in range(TB):
                routing_block(j)
        # ---- D phase: down-projections, combine, write out ----
        if not skip_dphase:
            for hb in range(HCH):
                dw_t = dpool.tile([P, FBN, 512], _BF16, tag="dw", name="dw_t")
                nc.sync.dma_start(dw_t[:], dwl_d[hb])
                sd_t = dpool.tile([P, SBN, 512], _BF16, tag="sd", name="sd_t")
                nc.sync.dma_start(sd_t[:], sdl_d[hb])
                for tb in range(TB):
                    tbs = slice(tb * P, (tb + 1) * P)
                    pe = mmp.tile([P, 512], _F32, tag="pg", bufs=4, name="pe")
                    for fb in range(FBN):
                        nc.tensor.matmul(
                            pe[:], aT[:, fb, tbs], dw_t[:, fb, :],
                            start=(fb == 0), stop=(fb == FBN - 1),
                        )
                    ps = mmp.tile([P, 512], _F32, tag="pu", bufs=3, name="ps")
                    for sb in range(SBN):
                        nc.tensor.matmul(
                            ps[:], ash[:, sb, tbs], sd_t[:, sb, :],
                            start=(sb == 0), stop=(sb == SBN - 1),
                        )
                    o = opool.tile([P, 512], _F32, tag="o", name="o")
                    nc.scalar.copy(o[:], ps[:])
                    # o = expert_psum * w_token + shared
                    nc.vector.scalar_tensor_tensor(
                        o[:], pe[:], w_all[:, tb : tb + 1], o[:],
                        _ALU.mult, _ALU.add,
                    )
                    nc.sync.dma_start(out_d[tbs, hb * 512:(hb + 1) * 512], o[:])


def build_program(repeat=1, **flags):
    nc = bacc.Bacc("TRN2", target_bir_lowering=False, debug=False)
    with tile.TileContext(nc) as tc:
        with ExitStack() as ctx:
            _build_body(ctx, tc, repeat=repeat, **flags)
    nc.compile()
    return nc


def _get_nc():
    global _CACHED_NC
    if _CACHED_NC is None:
        _CACHED_NC = build_program()
    return _CACHED_NC


def make_in_maps(inputs):
    """Host-side shard/layout prep: returns the 8 per-core input dicts."""
    h = np.asarray(inputs["hidden_states"], F32).reshape(T, H)
    hT = np.ascontiguousarray(h.T)                              # [H, T]
    hbf_in = np.ascontiguousarray(
        hT.reshape(KO, P, T).transpose(1, 0, 2).astype(BF16)
    )
    hf_in = np.ascontiguousarray(
        hT.reshape(KO, P, TB, P).transpose(2, 1, 0, 3)
    )
    gw8T = np.asarray(inputs["gate_weight"], F32).T             # [H, 8]
    gw8_in = np.ascontiguousarray(gw8T.reshape(KO, P, NEXP).transpose(1, 0, 2))

    gate_w = np.asarray(inputs["gate_w"], F32)
    up_w = np.asarray(inputs["up_w"], F32)
    down_w = np.asarray(inputs["down_w"], F32)
    sh_gate_w = np.asarray(inputs["sh_gate_w"], F32)
    sh_up_w = np.asarray(inputs["sh_up_w"], F32)
    sh_down_w = np.asarray(inputs["sh_down_w"], F32)

    in_maps = []
    for n in range(NEXP):
        # expert weights: [fb, p(h-inner), ko(h-outer), f'] layouts
        gw4 = gate_w[n].reshape(FBN, P, KO, P)       # (fb, f', ko, p)
        gwl_in = np.ascontiguousarray(gw4.transpose(0, 3, 2, 1).astype(BF16))
        uw4 = up_w[n].reshape(FBN, P, KO, P)
        uwl_in = np.ascontiguousarray(uw4.transpose(0, 3, 2, 1).astype(BF16))
        # shared expert slice, padded 352 -> 384 channels
        shg = np.zeros((FSLP, H), F32)
        shg[:FSL] = sh_gate_w[n * FSL : (n + 1) * FSL]
        sgl_in = np.ascontiguousarray(
            shg.reshape(SBN, P, KO, P).transpose(0, 3, 2, 1).astype(BF16)
        )
        shu = np.zeros((FSLP, H), F32)
        shu[:FSL] = sh_up_w[n * FSL : (n + 1) * FSL]
        sul_in = np.ascontiguousarray(
            shu.reshape(SBN, P, KO, P).transpose(0, 3, 2, 1).astype(BF16)
        )
        # down weights: [hb, p(f-inner), fb, h'] layouts
        dw4 = down_w[n].reshape(HCH, 512, FBN, P)    # (hb, h', fb, p)
        dwl_in = np.ascontiguousarray(dw4.transpose(0, 3, 2, 1).astype(BF16))
        sd = np.zeros((H, FSLP), F32)
        sd[:, :FSL] = sh_down_w[:, n * FSL : (n + 1) * FSL]
        sdl_in = np.ascontiguousarray(
            sd.reshape(HCH, 512, SBN, P).transpose(0, 3, 2, 1).astype(BF16)
        )
        esel_in = np.zeros((P, NEXP), F32)
        esel_in[:, n] = 1.0
        in_maps.append({
            "hbf": hbf_in, "hf": hf_in, "gw8": gw8_in, "esel": esel_in,
            "gwl": gwl_in, "uwl": uwl_in, "sgl": sgl_in, "sul": sul_in,
            "dwl": dwl_in, "sdl": sdl_in,
        })
    return in_maps


def run(inputs, trace=False, **kwargs):
    nc = _get_nc()
    in_maps = make_in_maps(inputs)
    res = run_bass_kernel_spmd(
        nc, in_maps, core_ids=list(range(NEXP)), trace=trace, **kwargs
    )
    total = res.results[0]["out"].astype(F32)
    for i in range(1, NEXP):
        total = total + res.results[i]["out"]
    return total.reshape(B, S, H), res


def kernel(**inputs):
    out, _ = run(inputs)
    return out



# revision 2
# speedup vs baseline: 2.2817x; 2.2817x over previous
"""DeepSeek-V2-style MoE kernel for 8 Trainium2 NeuronCores — sparse dispatch.

Sharding strategy:
  - Routing (gate matmul + softmax + group-limited top-2) runs on the host in
    fp32 numpy, mirroring the reference exactly. Only the top-2 experts per
    token contribute (combine weights of the rest are exactly zero), so each
    core computes its expert on just the tokens routed to it (gathered and
    zero-padded to a fixed capacity C=640; ~554 max at T=2048, K=2).
  - Core n runs routed expert n on its gathered tokens, scaled by the
    per-token combine weight, producing a [C, H] output the host scatters.
  - The always-on shared expert is 2D-sharded: 4-way over tokens x 2-way over
    the FS=2816 intermediate dim, so each core handles 512 tokens x 1408
    channels — the same shapes as the routed phase. Host adds core pairs.
  - If an expert ever exceeds capacity (not the case for the fixed harness
    input), the overflow tokens are computed exactly on the host in fp32.

All matmuls run in bf16 with fp32 PSUM accumulation, weight-stationary
(the PE pulls LDWEIGHTS ahead of in-flight matmuls, so per-(fb,ko) weight
loads hide under the 512-cycle token streams).
"""

from contextlib import ExitStack

import numpy as np
import ml_dtypes

import concourse.bass as bass
import concourse.tile as tile
from concourse import bacc, mybir
from concourse.bass_utils import run_bass_kernel_spmd

BF16 = ml_dtypes.bfloat16
F32 = np.float32

P = 128
B, S, H, F, FS, NEXP = 2, 1024, 2048, 1408, 2816, 8
T = B * S                      # 2048 tokens
TOP_K = 2
N_GROUP = 4
TOPK_GROUP = 2
KO = H // P                    # 16 contraction chunks over H
FBN = F // P                   # 11 intermediate-dim blocks of 128
C = 640                        # routed token capacity per expert
CB = C // P                    # 5 routed token blocks
TS = T // 4                    # 512 shared-expert tokens per core
SB = TS // P                   # 4 shared token blocks
HC = H // 512                  # 4 output chunks of 512

_ALU = mybir.AluOpType
_ACTF = mybir.ActivationFunctionType
_F32 = mybir.dt.float32
_BF16 = mybir.dt.bfloat16

_CACHED_NC = None


def _build_body(ctx, tc):
    nc = tc.nc
    xg_d = nc.dram_tensor("xg", [P, KO, C], _BF16, kind="ExternalInput").ap()
    xs_d = nc.dram_tensor("xs", [P, KO, TS], _BF16, kind="ExternalInput").ap()
    wg_d = nc.dram_tensor("wg", [P, FBN, KO, P], _BF16, kind="ExternalInput").ap()
    wu_d = nc.dram_tensor("wu", [P, FBN, KO, P], _BF16, kind="ExternalInput").ap()
    sg_d = nc.dram_tensor("sg", [P, FBN, KO, P], _BF16, kind="ExternalInput").ap()
    su_d = nc.dram_tensor("su", [P, FBN, KO, P], _BF16, kind="ExternalInput").ap()
    dw_d = nc.dram_tensor("dw", [P, FBN, H], _BF16, kind="ExternalInput").ap()
    sd_d = nc.dram_tensor("sd", [P, FBN, H], _BF16, kind="ExternalInput").ap()
    wr_d = nc.dram_tensor("wr", [P, CB], _F32, kind="ExternalInput").ap()
    outr_d = nc.dram_tensor("outr", [C, H], _F32, kind="ExternalOutput").ap()
    outs_d = nc.dram_tensor("outs", [TS, H], _F32, kind="ExternalOutput").ap()

    consts = ctx.enter_context(tc.tile_pool(name="consts", bufs=1))
    xpool = ctx.enter_context(tc.tile_pool(name="xpool", bufs=1))
    wpool = ctx.enter_context(tc.tile_pool(name="wpool", bufs=2))
    dpool = ctx.enter_context(tc.tile_pool(name="dpool", bufs=1))
    apool = ctx.enter_context(tc.tile_pool(name="apool", bufs=1))
    spool = ctx.enter_context(tc.tile_pool(name="spool", bufs=2))
    opool = ctx.enter_context(tc.tile_pool(name="opool", bufs=2))
    mmp = ctx.enter_context(tc.tile_pool(name="mmp", bufs=1, space="PSUM"))

    wr_sb = consts.tile([P, CB], _F32)
    nc.sync.dma_start(wr_sb[:], wr_d[:])

    xg_sb = xpool.tile([P, KO, C], _BF16)
    for i in range(4):
        ks = slice(i * 4, (i + 1) * 4)
        nc.sync.dma_start(xg_sb[:, ks, :], xg_d[:, ks, :])
    xs_sb = xpool.tile([P, KO, TS], _BF16)
    for i in range(4):
        ks = slice(i * 4, (i + 1) * 4)
        nc.sync.dma_start(xs_sb[:, ks, :], xs_d[:, ks, :])

    # down-proj weights: resident for their whole phase; DMA'd up front
    dw_sb = dpool.tile([P, FBN, H], _BF16)
    nc.sync.dma_start(dw_sb[:], dw_d[:])
    sd_sb = dpool.tile([P, FBN, H], _BF16)
    nc.sync.dma_start(sd_sb[:], sd_d[:])

    aT = apool.tile([P, FBN, C], _BF16)    # routed silu(g)*u, [f, tok]
    asT = apool.tile([P, FBN, TS], _BF16)  # shared silu(g)*u, [f, tok]

    def gu_phase(gsrc, usrc, x_sb, chunks, dst):
        """Gate/up projections + silu(g)*u for one FFN, weight-stationary.
        chunks: list of (offset, size, psum-tag-pair) token chunks."""
        for fb in range(FBN):
            wg_t = wpool.tile([P, KO, P], _BF16, tag="wg", name="wg_t")
            nc.sync.dma_start(wg_t[:], gsrc[:, fb])
            wu_t = wpool.tile([P, KO, P], _BF16, tag="wu", name="wu_t")
            nc.sync.dma_start(wu_t[:], usrc[:, fb])
            pgs = [mmp.tile([P, sz], _F32, tag=f"p{2*i}", bufs=2,
                            name=f"pg{i}") for i, (_, sz) in enumerate(chunks)]
            pus = [mmp.tile([P, sz], _F32, tag=f"p{2*i+1}", bufs=2,
                            name=f"pu{i}") for i, (_, sz) in enumerate(chunks)]
            for ko in range(KO):
                for i, (o, sz) in enumerate(chunks):
                    nc.tensor.matmul(
                        pgs[i][:], wg_t[:, ko, :], x_sb[:, ko, o:o + sz],
                        start=(ko == 0), stop=(ko == KO - 1),
                    )
            for ko in range(KO):
                for i, (o, sz) in enumerate(chunks):
                    nc.tensor.matmul(
                        pus[i][:], wu_t[:, ko, :], x_sb[:, ko, o:o + sz],
                        start=(ko == 0), stop=(ko == KO - 1),
                    )
            for i, (o, sz) in enumerate(chunks):
                sg = spool.tile([P, sz], _F32, tag="sg", name="sg")
                nc.scalar.activation(sg[:], pgs[i][:], _ACTF.Sigmoid)
                nc.vector.tensor_tensor(sg[:], sg[:], pgs[i][:], _ALU.mult)
                nc.vector.tensor_tensor(dst[:, fb, o:o + sz], sg[:],
                                        pus[i][:], _ALU.mult)

    def down_phase(a_sb, d_sb, ntb, out_d, scale):
        """Down-projection, activation-stationary (lhs = a[f, tok-block]),
        streaming the [f, H] weights as the moving operand."""
        for tb in range(ntb):
            tbs = slice(tb * P, (tb + 1) * P)
            pds = [mmp.tile([P, 512], _F32, tag=f"p{hc}", bufs=2,
                            name=f"pd{hc}") for hc in range(HC)]
            for fb in range(FBN):
                for hc in range(HC):
                    nc.tensor.matmul(
                        pds[hc][:], a_sb[:, fb, tbs],
                        d_sb[:, fb, hc * 512:(hc + 1) * 512],
                        start=(fb == 0), stop=(fb == FBN - 1),
                    )
            o = opool.tile([P, H], _F32, tag="o", name="o")
            for hc in range(HC):
                hs = slice(hc * 512, (hc + 1) * 512)
                if scale:
                    nc.vector.tensor_scalar_mul(o[:, hs], pds[hc][:],
                                                wr_sb[:, tb:tb + 1])
                else:
                    nc.scalar.copy(o[:, hs], pds[hc][:])
            nc.sync.dma_start(out_d[tbs, :], o[:])

    r_chunks = [(0, 512), (512, C - 512)]
    s_chunks = [(0, 512)]
    gu_phase(wg_d, wu_d, xg_sb, r_chunks, aT)
    down_phase(aT, dw_sb, CB, outr_d, scale=True)
    gu_phase(sg_d, su_d, xs_sb, s_chunks, asT)
    down_phase(asT, sd_sb, SB, outs_d, scale=False)


def build_program():
    nc = bacc.Bacc("TRN2", target_bir_lowering=False, debug=False)
    with tile.TileContext(nc) as tc:
        with ExitStack() as ctx:
            _build_body(ctx, tc)
    nc.compile()
    return nc


def _get_nc():
    global _CACHED_NC
    if _CACHED_NC is None:
        _CACHED_NC = build_program()
    return _CACHED_NC


def _route(h, gate_weight):
    """Mirror of the reference's softmax + group-limited top-2, numpy fp32."""
    logits = (h @ gate_weight.T).astype(F32)
    m = logits.max(-1, keepdims=True)
    e = np.exp(logits - m)
    scores = e / e.sum(-1, keepdims=True)                     # [T, N]
    E = NEXP // N_GROUP
    gs = scores.reshape(T, N_GROUP, E).max(-1)                # [T, G]
    gidx = np.argsort(-gs, axis=1, kind="stable")[:, :TOPK_GROUP]
    gmask = np.zeros((T, N_GROUP), F32)
    np.put_along_axis(gmask, gidx, 1.0, axis=1)
    masked = np.where(np.repeat(gmask, E, axis=1) > 0, scores, 0.0)
    ti = np.argsort(-masked, axis=1, kind="stable")[:, :TOP_K]  # [T, K]
    tw = np.take_along_axis(masked, ti, axis=1)               # [T, K]
    return tw, ti


def _wslab(w, half=None):
    """[F', H] row-major weight -> [P, FBN, KO, P] bf16 lhsT layout."""
    m = w if half is None else w[half * F:(half + 1) * F]
    return np.ascontiguousarray(
        m.reshape(FBN, P, KO, P).transpose(3, 0, 2, 1).astype(BF16))


def _dslab(w, half=None):
    """[H, F'] down weight -> [P, FBN, H] bf16 (f-inner, fb, h') layout."""
    m = w if half is None else w[:, half * F:(half + 1) * F]
    return np.ascontiguousarray(
        m.T.reshape(FBN, P, H).transpose(1, 0, 2).astype(BF16))


def _xslab(hT, pad_to):
    """[H, t] f32 column-slice of tokens -> [P, KO, pad_to] bf16."""
    t = hT.shape[1]
    out = np.zeros((P, KO, pad_to), BF16)
    out[:, :, :t] = hT.reshape(KO, P, t).transpose(1, 0, 2).astype(BF16)
    return out


def prepare(inputs):
    h = np.asarray(inputs["hidden_states"], F32).reshape(T, H)
    hT = np.ascontiguousarray(h.T)                            # [H, T]
    tw, ti = _route(h, np.asarray(inputs["gate_weight"], F32))

    gate_w = np.asarray(inputs["gate_w"], F32)
    up_w = np.asarray(inputs["up_w"], F32)
    down_w = np.asarray(inputs["down_w"], F32)

    # shared-expert shards, built once and referenced by multiple cores
    sgh = [_wslab(np.asarray(inputs["sh_gate_w"], F32), hn) for hn in (0, 1)]
    suh = [_wslab(np.asarray(inputs["sh_up_w"], F32), hn) for hn in (0, 1)]
    sdh = [_dslab(np.asarray(inputs["sh_down_w"], F32), hn) for hn in (0, 1)]
    xsq = [_xslab(hT[:, q * TS:(q + 1) * TS], TS) for q in range(4)]

    in_maps, gathers, overflows = [], [], []
    for n in range(NEXP):
        sel = np.nonzero(ti == n)
        idx = sel[0]                                          # token ids
        w = tw[sel[0], sel[1]]                                # combine weights
        if len(idx) > C:
            overflows.append((n, idx[C:], w[C:]))
            idx, w = idx[:C], w[:C]
        gathers.append((idx, len(idx)))
        wr = np.zeros(C, F32)
        wr[:len(idx)] = w
        in_maps.append({
            "xg": _xslab(hT[:, idx], C),
            "xs": xsq[n // 2],
            "wg": _wslab(gate_w[n]),
            "wu": _wslab(up_w[n]),
            "sg": sgh[n % 2],
            "su": suh[n % 2],
            "dw": _dslab(down_w[n]),
            "sd": sdh[n % 2],
            "wr": np.ascontiguousarray(wr.reshape(CB, P).T),
        })
    return in_maps, gathers, overflows


def _silu(x):
    return x / (1.0 + np.exp(-x))


def run(inputs, trace=False, **kwargs):
    nc = _get_nc()
    in_maps, gathers, overflows = prepare(inputs)
    res = run_bass_kernel_spmd(
        nc, in_maps, core_ids=list(range(NEXP)), trace=trace, **kwargs
    )
    out = np.empty((T, H), F32)
    for q in range(4):
        out[q * TS:(q + 1) * TS] = (res.results[2 * q]["outs"]
                                    + res.results[2 * q + 1]["outs"])
    for n in range(NEXP):
        idx, cnt = gathers[n]
        out[idx] += res.results[n]["outr"][:cnt]
    for n, idx, w in overflows:   # exact host fallback, normally empty
        x = np.asarray(inputs["hidden_states"], F32).reshape(T, H)[idx]
        g = x @ np.asarray(inputs["gate_w"][n], F32).T
        u = x @ np.asarray(inputs["up_w"][n], F32).T
        out[idx] += ((_silu(g) * u) @ np.asarray(inputs["down_w"][n], F32).T
                     * w[:, None])
    return out.reshape(B, S, H), res


def kernel(**inputs):
    out, _ = run(inputs)
    return out


# revision 4
# speedup vs baseline: 2.4013x; 1.0524x over previous
"""DeepSeek-V2-style MoE kernel for 8 Trainium2 NeuronCores — sparse dispatch.

Sharding strategy:
  - Routing (gate matmul + softmax + group-limited top-2) runs on the host in
    fp32 numpy, mirroring the reference exactly. Only the top-2 experts per
    token contribute (combine weights of the rest are exactly zero), so each
    core computes its expert on just the tokens routed to it (gathered and
    zero-padded to a fixed capacity C=640; ~554 max at T=2048, K=2).
  - Core n runs routed expert n on its gathered tokens, scaled by the
    per-token combine weight, producing a [C, H] output the host scatters.
  - The always-on shared expert is 2D-sharded: 4-way over tokens x 2-way over
    the FS=2816 intermediate dim, so each core handles 512 tokens x 1408
    channels — the same shapes as the routed phase. Host adds core pairs.
  - If an expert ever exceeds capacity (not the case for the fixed harness
    input), the overflow tokens are computed exactly on the host in fp32.

All matmuls run in bf16 with fp32 PSUM accumulation, weight-stationary
(the PE pulls LDWEIGHTS ahead of in-flight matmuls, so per-(fb,ko) weight
loads hide under the 512-cycle token streams).
"""

from contextlib import ExitStack

import numpy as np
import ml_dtypes

import concourse.bass as bass
import concourse.tile as tile
from concourse import bacc, mybir
from concourse.bass_utils import run_bass_kernel_spmd

BF16 = ml_dtypes.bfloat16
F32 = np.float32

P = 128
B, S, H, F, FS, NEXP = 2, 1024, 2048, 1408, 2816, 8
T = B * S                      # 2048 tokens
TOP_K = 2
N_GROUP = 4
TOPK_GROUP = 2
KO = H // P                    # 16 contraction chunks over H
FBN = F // P                   # 11 intermediate-dim blocks of 128
C = 640                        # routed token capacity per expert
CB = C // P                    # 5 routed token blocks
TS = T // 4                    # 512 shared-expert tokens per core
SB = TS // P                   # 4 shared token blocks
HC = H // 512                  # 4 output chunks of 512

_ALU = mybir.AluOpType
_ACTF = mybir.ActivationFunctionType
_F32 = mybir.dt.float32
_BF16 = mybir.dt.bfloat16

_CACHED_NC = None


def _build_body(ctx, tc):
    nc = tc.nc
    xg_d = nc.dram_tensor("xg", [P, KO, C], _BF16, kind="ExternalInput").ap()
    xs_d = nc.dram_tensor("xs", [P, KO, TS], _BF16, kind="ExternalInput").ap()
    wg_d = nc.dram_tensor("wg", [P, FBN, KO, P], _BF16, kind="ExternalInput").ap()
    wu_d = nc.dram_tensor("wu", [P, FBN, KO, P], _BF16, kind="ExternalInput").ap()
    sg_d = nc.dram_tensor("sg", [P, FBN, KO, P], _BF16, kind="ExternalInput").ap()
    su_d = nc.dram_tensor("su", [P, FBN, KO, P], _BF16, kind="ExternalInput").ap()
    dw_d = nc.dram_tensor("dw", [P, FBN, H], _BF16, kind="ExternalInput").ap()
    sd_d = nc.dram_tensor("sd", [P, FBN, H], _BF16, kind="ExternalInput").ap()
    wr_d = nc.dram_tensor("wr", [P, CB], _F32, kind="ExternalInput").ap()
    outr_d = nc.dram_tensor("outr", [C, H], _F32, kind="ExternalOutput").ap()
    outs_d = nc.dram_tensor("outs", [TS, H], _F32, kind="ExternalOutput").ap()

    consts = ctx.enter_context(tc.tile_pool(name="consts", bufs=1))
    xpool = ctx.enter_context(tc.tile_pool(name="xpool", bufs=1))
    wpool = ctx.enter_context(tc.tile_pool(name="wpool", bufs=2))
    dpool = ctx.enter_context(tc.tile_pool(name="dpool", bufs=1))
    apool = ctx.enter_context(tc.tile_pool(name="apool", bufs=1))
    spool = ctx.enter_context(tc.tile_pool(name="spool", bufs=2))
    opool = ctx.enter_context(tc.tile_pool(name="opool", bufs=2))
    mmp = ctx.enter_context(tc.tile_pool(name="mmp", bufs=1, space="PSUM"))

    # Bulk loads go on the scalar (Activation) HWDGE queue so the sync
    # queue's first transfer is the fb0 gate-weight tile the PE waits on.
    wr_sb = consts.tile([P, CB], _F32)
    nc.scalar.dma_start(wr_sb[:], wr_d[:])

    xg_sb = xpool.tile([P, KO, C], _BF16)
    for i in range(4):
        ks = slice(i * 4, (i + 1) * 4)
        nc.scalar.dma_start(xg_sb[:, ks, :], xg_d[:, ks, :])
    # down-proj weights: resident for their whole phase; DMA'd up front
    dw_sb = dpool.tile([P, FBN, H], _BF16)
    nc.scalar.dma_start(dw_sb[:], dw_d[:])
    xs_sb = xpool.tile([P, KO, TS], _BF16)
    for i in range(4):
        ks = slice(i * 4, (i + 1) * 4)
        nc.scalar.dma_start(xs_sb[:, ks, :], xs_d[:, ks, :])
    sd_sb = dpool.tile([P, FBN, H], _BF16)
    nc.scalar.dma_start(sd_sb[:], sd_d[:])

    aT = apool.tile([P, FBN, C], _BF16)    # routed silu(g)*u, [f, tok]
    asT = apool.tile([P, FBN, TS], _BF16)  # shared silu(g)*u, [f, tok]

    def gu_phase(gsrc, usrc, x_sb, chunks, dst):
        """Gate/up projections + silu(g)*u for one FFN, weight-stationary.
        chunks: list of (offset, size, psum-tag-pair) token chunks."""
        for fb in range(FBN):
            wg_t = wpool.tile([P, KO, P], _BF16, tag="wg", name="wg_t")
            nc.sync.dma_start(wg_t[:], gsrc[:, fb])
            wu_t = wpool.tile([P, KO, P], _BF16, tag="wu", name="wu_t")
            nc.sync.dma_start(wu_t[:], usrc[:, fb])
            pgs = [mmp.tile([P, sz], _F32, tag=f"p{2*i}", bufs=2,
                            name=f"pg{i}") for i, (_, sz) in enumerate(chunks)]
            pus = [mmp.tile([P, sz], _F32, tag=f"p{2*i+1}", bufs=2,
                            name=f"pu{i}") for i, (_, sz) in enumerate(chunks)]
            for ko in range(KO):
                for i, (o, sz) in enumerate(chunks):
                    nc.tensor.matmul(
                        pgs[i][:], wg_t[:, ko, :], x_sb[:, ko, o:o + sz],
                        start=(ko == 0), stop=(ko == KO - 1),
                    )
            for ko in range(KO):
                for i, (o, sz) in enumerate(chunks):
                    nc.tensor.matmul(
                        pus[i][:], wu_t[:, ko, :], x_sb[:, ko, o:o + sz],
                        start=(ko == 0), stop=(ko == KO - 1),
                    )
            for i, (o, sz) in enumerate(chunks):
                sg = spool.tile([P, sz], _F32, tag="sg", name="sg")
                nc.scalar.activation(sg[:], pgs[i][:], _ACTF.Sigmoid)
                nc.vector.tensor_tensor(sg[:], sg[:], pgs[i][:], _ALU.mult)
                nc.vector.tensor_tensor(dst[:, fb, o:o + sz], sg[:],
                                        pus[i][:], _ALU.mult)

    def down_phase(a_sb, d_sb, ntb, out_d, scale):
        """Down-projection, activation-stationary (lhs = a[f, tok-block]),
        streaming the [f, H] weights as the moving operand."""
        for tb in range(ntb):
            tbs = slice(tb * P, (tb + 1) * P)
            pds = [mmp.tile([P, 512], _F32, tag=f"p{hc}", bufs=2,
                            name=f"pd{hc}") for hc in range(HC)]
            for fb in range(FBN):
                for hc in range(HC):
                    nc.tensor.matmul(
                        pds[hc][:], a_sb[:, fb, tbs],
                        d_sb[:, fb, hc * 512:(hc + 1) * 512],
                        start=(fb == 0), stop=(fb == FBN - 1),
                    )
            o = opool.tile([P, H], _F32, tag="o", bufs=3, name="o")
            for hc in range(HC):
                hs = slice(hc * 512, (hc + 1) * 512)
                if scale:
                    nc.vector.tensor_scalar_mul(o[:, hs], pds[hc][:],
                                                wr_sb[:, tb:tb + 1])
                else:
                    nc.scalar.copy(o[:, hs], pds[hc][:])
                nc.scalar.dma_start(out_d[tbs, hs], o[:, hs])

    r_chunks = [(0, 512), (512, C - 512)]
    s_chunks = [(0, 512)]
    gu_phase(wg_d, wu_d, xg_sb, r_chunks, aT)
    down_phase(aT, dw_sb, CB, outr_d, scale=True)
    gu_phase(sg_d, su_d, xs_sb, s_chunks, asT)
    down_phase(asT, sd_sb, SB, outs_d, scale=False)


def build_program():
    nc = bacc.Bacc("TRN2", target_bir_lowering=False, debug=False)
    with tile.TileContext(nc) as tc:
        with ExitStack() as ctx:
            _build_body(ctx, tc)
    nc.compile()
    return nc


def _get_nc():
    global _CACHED_NC
    if _CACHED_NC is None:
        _CACHED_NC = build_program()
    return _CACHED_NC


def _route(h, gate_weight):
    """Mirror of the reference's softmax + group-limited top-2, numpy fp32."""
    logits = (h @ gate_weight.T).astype(F32)
    m = logits.max(-1, keepdims=True)
    e = np.exp(logits - m)
    scores = e / e.sum(-1, keepdims=True)                     # [T, N]
    E = NEXP // N_GROUP
    gs = scores.reshape(T, N_GROUP, E).max(-1)                # [T, G]
    gidx = np.argsort(-gs, axis=1, kind="stable")[:, :TOPK_GROUP]
    gmask = np.zeros((T, N_GROUP), F32)
    np.put_along_axis(gmask, gidx, 1.0, axis=1)
    masked = np.where(np.repeat(gmask, E, axis=1) > 0, scores, 0.0)
    ti = np.argsort(-masked, axis=1, kind="stable")[:, :TOP_K]  # [T, K]
    tw = np.take_along_axis(masked, ti, axis=1)               # [T, K]
    return tw, ti


def _wslab(w, half=None):
    """[F', H] row-major weight -> [P, FBN, KO, P] bf16 lhsT layout."""
    m = w if half is None else w[half * F:(half + 1) * F]
    return np.ascontiguousarray(
        m.reshape(FBN, P, KO, P).transpose(3, 0, 2, 1).astype(BF16))


def _dslab(w, half=None):
    """[H, F'] down weight -> [P, FBN, H] bf16 (f-inner, fb, h') layout."""
    m = w if half is None else w[:, half * F:(half + 1) * F]
    return np.ascontiguousarray(
        m.T.reshape(FBN, P, H).transpose(1, 0, 2).astype(BF16))


def _xslab(hT, pad_to):
    """[H, t] f32 column-slice of tokens -> [P, KO, pad_to] bf16."""
    t = hT.shape[1]
    out = np.zeros((P, KO, pad_to), BF16)
    out[:, :, :t] = hT.reshape(KO, P, t).transpose(1, 0, 2).astype(BF16)
    return out


def prepare(inputs):
    h = np.asarray(inputs["hidden_states"], F32).reshape(T, H)
    hT = np.ascontiguousarray(h.T)                            # [H, T]
    tw, ti = _route(h, np.asarray(inputs["gate_weight"], F32))

    gate_w = np.asarray(inputs["gate_w"], F32)
    up_w = np.asarray(inputs["up_w"], F32)
    down_w = np.asarray(inputs["down_w"], F32)

    # shared-expert shards, built once and referenced by multiple cores
    sgh = [_wslab(np.asarray(inputs["sh_gate_w"], F32), hn) for hn in (0, 1)]
    suh = [_wslab(np.asarray(inputs["sh_up_w"], F32), hn) for hn in (0, 1)]
    sdh = [_dslab(np.asarray(inputs["sh_down_w"], F32), hn) for hn in (0, 1)]
    xsq = [_xslab(hT[:, q * TS:(q + 1) * TS], TS) for q in range(4)]

    in_maps, gathers, overflows = [], [], []
    for n in range(NEXP):
        sel = np.nonzero(ti == n)
        idx = sel[0]                                          # token ids
        w = tw[sel[0], sel[1]]                                # combine weights
        if len(idx) > C:
            overflows.append((n, idx[C:], w[C:]))
            idx, w = idx[:C], w[:C]
        gathers.append((idx, len(idx)))
        wr = np.zeros(C, F32)
        wr[:len(idx)] = w
        in_maps.append({
            "xg": _xslab(hT[:, idx], C),
            "xs": xsq[n // 2],
            "wg": _wslab(gate_w[n]),
            "wu": _wslab(up_w[n]),
            "sg": sgh[n % 2],
            "su": suh[n % 2],
            "dw": _dslab(down_w[n]),
            "sd": sdh[n % 2],
            "wr": np.ascontiguousarray(wr.reshape(CB, P).T),
        })
    return in_maps, gathers, overflows


def _silu(x):
    return x / (1.0 + np.exp(-x))


def run(inputs, trace=False, **kwargs):
    nc = _get_nc()
    in_maps, gathers, overflows = prepare(inputs)
    res = run_bass_kernel_spmd(
        nc, in_maps, core_ids=list(range(NEXP)), trace=trace, **kwargs
    )
    out = np.empty((T, H), F32)
    for q in range(4):
        out[q * TS:(q + 1) * TS] = (res.results[2 * q]["outs"]
                                    + res.results[2 * q + 1]["outs"])
    for n in range(NEXP):
        idx, cnt = gathers[n]
        out[idx] += res.results[n]["outr"][:cnt]
    for n, idx, w in overflows:   # exact host fallback, normally empty
        x = np.asarray(inputs["hidden_states"], F32).reshape(T, H)[idx]
        g = x @ np.asarray(inputs["gate_w"][n], F32).T
        u = x @ np.asarray(inputs["up_w"][n], F32).T
        out[idx] += ((_silu(g) * u) @ np.asarray(inputs["down_w"][n], F32).T
                     * w[:, None])
    return out.reshape(B, S, H), res


def kernel(**inputs):
    out, _ = run(inputs)
    return out


# revision 8
# speedup vs baseline: 2.4757x; 1.0310x over previous
"""DeepSeek-V2-style MoE kernel for 8 Trainium2 NeuronCores — sparse dispatch.

Sharding strategy:
  - Routing (gate matmul + softmax + group-limited top-2) runs on the host in
    fp32 numpy, mirroring the reference exactly. Only the top-2 experts per
    token contribute (combine weights of the rest are exactly zero), so each
    core computes its expert on just the tokens routed to it (gathered and
    zero-padded to a fixed capacity C=640; ~554 max at T=2048, K=2).
  - Core n runs routed expert n on its gathered tokens, scaled by the
    per-token combine weight, producing a [C, H] output the host scatters.
  - The always-on shared expert is 2D-sharded: 4-way over tokens x 2-way over
    the FS=2816 intermediate dim, so each core handles 512 tokens x 1408
    channels — the same shapes as the routed phase. Host adds core pairs.
  - If an expert ever exceeds capacity (not the case for the fixed harness
    input), the overflow tokens are computed exactly on the host in fp32.

All matmuls run in bf16 with fp32 PSUM accumulation, weight-stationary
(the PE pulls LDWEIGHTS ahead of in-flight matmuls, so per-(fb,ko) weight
loads hide under the 512-cycle token streams).
"""

from contextlib import ExitStack

import numpy as np
import ml_dtypes

import concourse.bass as bass
import concourse.tile as tile
from concourse import bacc, mybir
from concourse.bass_utils import run_bass_kernel_spmd

BF16 = ml_dtypes.bfloat16
F32 = np.float32

P = 128
B, S, H, F, FS, NEXP = 2, 1024, 2048, 1408, 2816, 8
T = B * S                      # 2048 tokens
TOP_K = 2
N_GROUP = 4
TOPK_GROUP = 2
KO = H // P                    # 16 contraction chunks over H
FBN = F // P                   # 11 intermediate-dim blocks of 128
C = 640                        # routed token capacity per expert
CB = C // P                    # 5 routed token blocks
TS = T // 4                    # 512 shared-expert tokens per core
SB = TS // P                   # 4 shared token blocks
HC = H // 512                  # 4 output chunks of 512

_ALU = mybir.AluOpType
_ACTF = mybir.ActivationFunctionType
_F32 = mybir.dt.float32
_BF16 = mybir.dt.bfloat16

_CACHED_NC = None


def _build_body(ctx, tc):
    nc = tc.nc
    xg_d = nc.dram_tensor("xg", [P, KO, C], _BF16, kind="ExternalInput").ap()
    xs_d = nc.dram_tensor("xs", [P, KO, TS], _BF16, kind="ExternalInput").ap()
    wg_d = nc.dram_tensor("wg", [P, FBN, KO, P], _BF16, kind="ExternalInput").ap()
    wu_d = nc.dram_tensor("wu", [P, FBN, KO, P], _BF16, kind="ExternalInput").ap()
    sg_d = nc.dram_tensor("sg", [P, FBN, KO, P], _BF16, kind="ExternalInput").ap()
    su_d = nc.dram_tensor("su", [P, FBN, KO, P], _BF16, kind="ExternalInput").ap()
    dw_d = nc.dram_tensor("dw", [P, FBN, H], _BF16, kind="ExternalInput").ap()
    sd_d = nc.dram_tensor("sd", [P, FBN, H], _BF16, kind="ExternalInput").ap()
    wr_d = nc.dram_tensor("wr", [P, CB], _F32, kind="ExternalInput").ap()
    outr_d = nc.dram_tensor("outr", [C, H], _F32, kind="ExternalOutput").ap()
    outs_d = nc.dram_tensor("outs", [TS, H], _F32, kind="ExternalOutput").ap()

    consts = ctx.enter_context(tc.tile_pool(name="consts", bufs=1))
    xpool = ctx.enter_context(tc.tile_pool(name="xpool", bufs=1))
    wpool = ctx.enter_context(tc.tile_pool(name="wpool", bufs=2))
    dpool = ctx.enter_context(tc.tile_pool(name="dpool", bufs=1))
    apool = ctx.enter_context(tc.tile_pool(name="apool", bufs=1))
    spool = ctx.enter_context(tc.tile_pool(name="spool", bufs=2))
    opool = ctx.enter_context(tc.tile_pool(name="opool", bufs=2))
    mmp = ctx.enter_context(tc.tile_pool(name="mmp", bufs=1, space="PSUM"))

    # Bulk loads go on the scalar (Activation) HWDGE queue so the sync
    # queue's first transfer is the fb0 gate-weight tile the PE waits on.
    wr_sb = consts.tile([P, CB], _F32)
    nc.scalar.dma_start(wr_sb[:], wr_d[:])

    xg_sb = xpool.tile([P, KO, C], _BF16)
    nc.scalar.dma_start(xg_sb[:, 0:4, :], xg_d[:, 0:4, :])
    nc.scalar.dma_start(xg_sb[:, 4:KO, :], xg_d[:, 4:KO, :])
    # down-proj weights: resident for their whole phase; DMA'd up front
    dw_sb = dpool.tile([P, FBN, H], _BF16)
    nc.scalar.dma_start(dw_sb[:], dw_d[:])
    xs_sb = xpool.tile([P, KO, TS], _BF16)
    nc.scalar.dma_start(xs_sb[:], xs_d[:])
    sd_sb = dpool.tile([P, FBN, H], _BF16)

    aT = apool.tile([P, FBN, C], _BF16)    # routed silu(g)*u, [f, tok]
    asT = apool.tile([P, FBN, TS], _BF16)  # shared silu(g)*u, [f, tok]

    FBG = 2                       # weight-stream DMA group: 2 fb per op

    def gu_phase(gsrc, usrc, x_sb, chunks, dst):
        """Gate/up projections + silu(g)*u for one FFN, weight-stationary.
        Weight tiles stream in 2-fb groups (1 MB DMA ops amortize the ~2us
        per-op completion latency)."""
        for g0 in range(0, FBN, FBG):
            gsz = min(FBG, FBN - g0)
            wg_t = wpool.tile([P, FBG, KO, P], _BF16, tag="wg", name="wg_t")
            nc.sync.dma_start(wg_t[:, :gsz], gsrc[:, g0:g0 + gsz])
            wu_t = wpool.tile([P, FBG, KO, P], _BF16, tag="wu", name="wu_t")
            nc.sync.dma_start(wu_t[:, :gsz], usrc[:, g0:g0 + gsz])
            for j in range(gsz):
                fb = g0 + j
                pgs = [mmp.tile([P, sz], _F32, tag=f"p{2*i}", bufs=2,
                                name=f"pg{i}")
                       for i, (_, sz) in enumerate(chunks)]
                pus = [mmp.tile([P, sz], _F32, tag=f"p{2*i+1}", bufs=2,
                                name=f"pu{i}")
                       for i, (_, sz) in enumerate(chunks)]
                for ko in range(KO):
                    for i, (o, sz) in enumerate(chunks):
                        nc.tensor.matmul(
                            pgs[i][:], wg_t[:, j, ko, :],
                            x_sb[:, ko, o:o + sz],
                            start=(ko == 0), stop=(ko == KO - 1),
                        )
                for ko in range(KO):
                    for i, (o, sz) in enumerate(chunks):
                        nc.tensor.matmul(
                            pus[i][:], wu_t[:, j, ko, :],
                            x_sb[:, ko, o:o + sz],
                            start=(ko == 0), stop=(ko == KO - 1),
                        )
                for i, (o, sz) in enumerate(chunks):
                    sg = spool.tile([P, sz], _F32, tag="sg", name="sg")
                    nc.scalar.activation(sg[:], pgs[i][:], _ACTF.Sigmoid)
                    nc.vector.tensor_tensor(sg[:], sg[:], pgs[i][:],
                                            _ALU.mult)
                    nc.vector.tensor_tensor(dst[:, fb, o:o + sz], sg[:],
                                            pus[i][:], _ALU.mult)

    def down_phase(a_sb, d_sb, ntb, out_d, scale, split_out=False):
        """Down-projection, activation-stationary (lhs = a[f, tok-block]),
        streaming the [f, H] weights as the moving operand."""
        for tb in range(ntb):
            tbs = slice(tb * P, (tb + 1) * P)
            pds = [mmp.tile([P, 512], _F32, tag=f"p{hc}", bufs=2,
                            name=f"pd{hc}") for hc in range(HC)]
            for fb in range(FBN):
                for hc in range(HC):
                    nc.tensor.matmul(
                        pds[hc][:], a_sb[:, fb, tbs],
                        d_sb[:, fb, hc * 512:(hc + 1) * 512],
                        start=(fb == 0), stop=(fb == FBN - 1),
                    )
            o = opool.tile([P, H], _F32, tag="o", name="o")
            for hc in range(HC):
                hs = slice(hc * 512, (hc + 1) * 512)
                if scale:
                    nc.vector.tensor_scalar_mul(o[:, hs], pds[hc][:],
                                                wr_sb[:, tb:tb + 1])
                else:
                    nc.scalar.copy(o[:, hs], pds[hc][:])
                if split_out:
                    nc.scalar.dma_start(out_d[tbs, hs], o[:, hs])
            if not split_out:
                nc.scalar.dma_start(out_d[tbs, :], o[:])

    r_chunks = [(0, 512), (512, C - 512)]
    s_chunks = [(0, 512)]
    gu_phase(wg_d, wu_d, xg_sb, r_chunks, aT)
    # sd arrives on the scalar queue behind the routed sigmoids, i.e. its
    # transfer starts ~when the routed down phase begins — well before use
    nc.scalar.dma_start(sd_sb[:], sd_d[:])
    down_phase(aT, dw_sb, CB, outr_d, scale=True)
    gu_phase(sg_d, su_d, xs_sb, s_chunks, asT)
    down_phase(asT, sd_sb, SB, outs_d, scale=False, split_out=True)


def build_program():
    nc = bacc.Bacc("TRN2", target_bir_lowering=False, debug=False)
    with tile.TileContext(nc) as tc:
        with ExitStack() as ctx:
            _build_body(ctx, tc)
    nc.compile()
    return nc


def _get_nc():
    global _CACHED_NC
    if _CACHED_NC is None:
        _CACHED_NC = build_program()
    return _CACHED_NC


def _route(h, gate_weight):
    """Mirror of the reference's softmax + group-limited top-2, numpy fp32."""
    logits = (h @ gate_weight.T).astype(F32)
    m = logits.max(-1, keepdims=True)
    e = np.exp(logits - m)
    scores = e / e.sum(-1, keepdims=True)                     # [T, N]
    E = NEXP // N_GROUP
    gs = scores.reshape(T, N_GROUP, E).max(-1)                # [T, G]
    gidx = np.argsort(-gs, axis=1, kind="stable")[:, :TOPK_GROUP]
    gmask = np.zeros((T, N_GROUP), F32)
    np.put_along_axis(gmask, gidx, 1.0, axis=1)
    masked = np.where(np.repeat(gmask, E, axis=1) > 0, scores, 0.0)
    ti = np.argsort(-masked, axis=1, kind="stable")[:, :TOP_K]  # [T, K]
    tw = np.take_along_axis(masked, ti, axis=1)               # [T, K]
    return tw, ti


def _wslab(w, half=None):
    """[F', H] row-major weight -> [P, FBN, KO, P] bf16 lhsT layout."""
    m = w if half is None else w[half * F:(half + 1) * F]
    return np.ascontiguousarray(
        m.reshape(FBN, P, KO, P).transpose(3, 0, 2, 1).astype(BF16))


def _dslab(w, half=None):
    """[H, F'] down weight -> [P, FBN, H] bf16 (f-inner, fb, h') layout."""
    m = w if half is None else w[:, half * F:(half + 1) * F]
    return np.ascontiguousarray(
        m.T.reshape(FBN, P, H).transpose(1, 0, 2).astype(BF16))


def _xslab(hT, pad_to):
    """[H, t] f32 column-slice of tokens -> [P, KO, pad_to] bf16."""
    t = hT.shape[1]
    out = np.zeros((P, KO, pad_to), BF16)
    out[:, :, :t] = hT.reshape(KO, P, t).transpose(1, 0, 2).astype(BF16)
    return out


def prepare(inputs):
    h = np.asarray(inputs["hidden_states"], F32).reshape(T, H)
    hT = np.ascontiguousarray(h.T)                            # [H, T]
    tw, ti = _route(h, np.asarray(inputs["gate_weight"], F32))

    gate_w = np.asarray(inputs["gate_w"], F32)
    up_w = np.asarray(inputs["up_w"], F32)
    down_w = np.asarray(inputs["down_w"], F32)

    # shared-expert shards, built once and referenced by multiple cores
    sgh = [_wslab(np.asarray(inputs["sh_gate_w"], F32), hn) for hn in (0, 1)]
    suh = [_wslab(np.asarray(inputs["sh_up_w"], F32), hn) for hn in (0, 1)]
    sdh = [_dslab(np.asarray(inputs["sh_down_w"], F32), hn) for hn in (0, 1)]
    xsq = [_xslab(hT[:, q * TS:(q + 1) * TS], TS) for q in range(4)]

    in_maps, gathers, overflows = [], [], []
    for n in range(NEXP):
        sel = np.nonzero(ti == n)
        idx = sel[0]                                          # token ids
        w = tw[sel[0], sel[1]]                                # combine weights
        if len(idx) > C:
            overflows.append((n, idx[C:], w[C:]))
            idx, w = idx[:C], w[:C]
        gathers.append((idx, len(idx)))
        wr = np.zeros(C, F32)
        wr[:len(idx)] = w
        in_maps.append({
            "xg": _xslab(hT[:, idx], C),
            "xs": xsq[n // 2],
            "wg": _wslab(gate_w[n]),
            "wu": _wslab(up_w[n]),
            "sg": sgh[n % 2],
            "su": suh[n % 2],
            "dw": _dslab(down_w[n]),
            "sd": sdh[n % 2],
            "wr": np.ascontiguousarray(wr.reshape(CB, P).T),
        })
    return in_maps, gathers, overflows


def _silu(x):
    return x / (1.0 + np.exp(-x))


def run(inputs, trace=False, **kwargs):
    nc = _get_nc()
    in_maps, gathers, overflows = prepare(inputs)
    res = run_bass_kernel_spmd(
        nc, in_maps, core_ids=list(range(NEXP)), trace=trace, **kwargs
    )
    out = np.empty((T, H), F32)
    for q in range(4):
        out[q * TS:(q + 1) * TS] = (res.results[2 * q]["outs"]
                                    + res.results[2 * q + 1]["outs"])
    for n in range(NEXP):
        idx, cnt = gathers[n]
        out[idx] += res.results[n]["outr"][:cnt]
    for n, idx, w in overflows:   # exact host fallback, normally empty
        x = np.asarray(inputs["hidden_states"], F32).reshape(T, H)[idx]
        g = x @ np.asarray(inputs["gate_w"][n], F32).T
        u = x @ np.asarray(inputs["up_w"][n], F32).T
        out[idx] += ((_silu(g) * u) @ np.asarray(inputs["down_w"][n], F32).T
                     * w[:, None])
    return out.reshape(B, S, H), res


def kernel(**inputs):
    out, _ = run(inputs)
    return out


# revision 12
# speedup vs baseline: 2.4818x; 1.0025x over previous
"""DeepSeek-V2-style MoE kernel for 8 Trainium2 NeuronCores — sparse dispatch.

Sharding strategy:
  - Routing (gate matmul + softmax + group-limited top-2) runs on the host in
    fp32 numpy, mirroring the reference exactly. Only the top-2 experts per
    token contribute (combine weights of the rest are exactly zero), so each
    core computes its expert on just the tokens routed to it (gathered and
    zero-padded to a fixed capacity C=640; ~554 max at T=2048, K=2).
  - Core n runs routed expert n on its gathered tokens, scaled by the
    per-token combine weight, producing a [C, H] output the host scatters.
  - The always-on shared expert is 2D-sharded: 4-way over tokens x 2-way over
    the FS=2816 intermediate dim, so each core handles 512 tokens x 1408
    channels — the same shapes as the routed phase. Host adds core pairs.
  - If an expert ever exceeds capacity (not the case for the fixed harness
    input), the overflow tokens are computed exactly on the host in fp32.

All matmuls run in bf16 with fp32 PSUM accumulation, weight-stationary
(the PE pulls LDWEIGHTS ahead of in-flight matmuls, so per-(fb,ko) weight
loads hide under the 512-cycle token streams).
"""

from contextlib import ExitStack

import numpy as np
import ml_dtypes

import concourse.bass as bass
import concourse.tile as tile
from concourse import bacc, mybir
from concourse.bass_utils import run_bass_kernel_spmd

BF16 = ml_dtypes.bfloat16
F32 = np.float32

P = 128
B, S, H, F, FS, NEXP = 2, 1024, 2048, 1408, 2816, 8
T = B * S                      # 2048 tokens
TOP_K = 2
N_GROUP = 4
TOPK_GROUP = 2
KO = H // P                    # 16 contraction chunks over H
FBN = F // P                   # 11 intermediate-dim blocks of 128
C = 640                        # routed token capacity per expert
CB = C // P                    # 5 routed token blocks
TS = T // 4                    # 512 shared-expert tokens per core
SB = TS // P                   # 4 shared token blocks
HC = H // 512                  # 4 output chunks of 512

_ALU = mybir.AluOpType
_ACTF = mybir.ActivationFunctionType
_F32 = mybir.dt.float32
_BF16 = mybir.dt.bfloat16

_CACHED_NC = None


def _build_body(ctx, tc):
    nc = tc.nc
    xg_d = nc.dram_tensor("xg", [P, KO, C], _BF16, kind="ExternalInput").ap()
    xs_d = nc.dram_tensor("xs", [P, KO, TS], _BF16, kind="ExternalInput").ap()
    wg_d = nc.dram_tensor("wg", [P, FBN, KO, P], _BF16, kind="ExternalInput").ap()
    wu_d = nc.dram_tensor("wu", [P, FBN, KO, P], _BF16, kind="ExternalInput").ap()
    sg_d = nc.dram_tensor("sg", [P, FBN, KO, P], _BF16, kind="ExternalInput").ap()
    su_d = nc.dram_tensor("su", [P, FBN, KO, P], _BF16, kind="ExternalInput").ap()
    dw_d = nc.dram_tensor("dw", [P, FBN, H], _BF16, kind="ExternalInput").ap()
    sd_d = nc.dram_tensor("sd", [P, FBN, H], _BF16, kind="ExternalInput").ap()
    wr_d = nc.dram_tensor("wr", [P, CB], _F32, kind="ExternalInput").ap()
    outr_d = nc.dram_tensor("outr", [C, H], _F32, kind="ExternalOutput").ap()
    outs_d = nc.dram_tensor("outs", [TS, H], _F32, kind="ExternalOutput").ap()

    consts = ctx.enter_context(tc.tile_pool(name="consts", bufs=1))
    xpool = ctx.enter_context(tc.tile_pool(name="xpool", bufs=1))
    wpool = ctx.enter_context(tc.tile_pool(name="wpool", bufs=2))
    dpool = ctx.enter_context(tc.tile_pool(name="dpool", bufs=1))
    apool = ctx.enter_context(tc.tile_pool(name="apool", bufs=1))
    spool = ctx.enter_context(tc.tile_pool(name="spool", bufs=2))
    opool = ctx.enter_context(tc.tile_pool(name="opool", bufs=2))
    mmp = ctx.enter_context(tc.tile_pool(name="mmp", bufs=1, space="PSUM"))

    # Bulk loads go on the scalar (Activation) HWDGE queue so the sync
    # queue's first transfer is the fb0 gate-weight tile the PE waits on.
    wr_sb = consts.tile([P, CB], _F32)
    nc.scalar.dma_start(wr_sb[:], wr_d[:])

    xg_sb = xpool.tile([P, KO, C], _BF16)
    nc.scalar.dma_start(xg_sb[:, 0:4, :], xg_d[:, 0:4, :])
    nc.scalar.dma_start(xg_sb[:, 4:KO, :], xg_d[:, 4:KO, :])
    # down-proj weights: dw and sd share one slot (sd's DMA is deferred and
    # waits for the routed down phase to finish reading dw)
    dw_sb = dpool.tile([P, FBN, H], _BF16, tag="dwn", name="dw_sb")
    nc.scalar.dma_start(dw_sb[:], dw_d[:])
    xs_sb = xpool.tile([P, KO, TS], _BF16)
    nc.scalar.dma_start(xs_sb[:], xs_d[:])

    aT = apool.tile([P, FBN, C], _BF16)    # routed silu(g)*u, [f, tok]
    asT = apool.tile([P, FBN, TS], _BF16)  # shared silu(g)*u, [f, tok]

    FBG = 2                       # weight-stream DMA group: 2 fb per op
    GROUPS = [(0, 1)] + [(g, min(FBG, FBN - g)) for g in range(1, FBN, FBG)]

    def gu_phase(gsrc, usrc, x_sb, chunks, dst):
        """Gate/up projections + silu(g)*u for one FFN, weight-stationary.
        Weight tiles stream in 2-fb groups (1 MB DMA ops amortize the ~2us
        per-op completion latency); the first group is a single fb so the
        PE can start sooner."""
        for g0, gsz in GROUPS:
            wg_t = wpool.tile([P, FBG, KO, P], _BF16, tag="wg", bufs=3,
                              name="wg_t")
            nc.sync.dma_start(wg_t[:, :gsz], gsrc[:, g0:g0 + gsz])
            wu_t = wpool.tile([P, FBG, KO, P], _BF16, tag="wu", bufs=3,
                              name="wu_t")
            nc.sync.dma_start(wu_t[:, :gsz], usrc[:, g0:g0 + gsz])
            for j in range(gsz):
                fb = g0 + j
                pgs = [mmp.tile([P, sz], _F32, tag=f"p{2*i}", bufs=2,
                                name=f"pg{i}")
                       for i, (_, sz) in enumerate(chunks)]
                pus = [mmp.tile([P, sz], _F32, tag=f"p{2*i+1}", bufs=2,
                                name=f"pu{i}")
                       for i, (_, sz) in enumerate(chunks)]
                for ko in range(KO):
                    for i, (o, sz) in enumerate(chunks):
                        nc.tensor.matmul(
                            pgs[i][:], wg_t[:, j, ko, :],
                            x_sb[:, ko, o:o + sz],
                            start=(ko == 0), stop=(ko == KO - 1),
                        )
                for ko in range(KO):
                    for i, (o, sz) in enumerate(chunks):
                        nc.tensor.matmul(
                            pus[i][:], wu_t[:, j, ko, :],
                            x_sb[:, ko, o:o + sz],
                            start=(ko == 0), stop=(ko == KO - 1),
                        )
                for i, (o, sz) in enumerate(chunks):
                    sg = spool.tile([P, sz], _F32, tag="sg", name="sg")
                    nc.scalar.activation(sg[:], pgs[i][:], _ACTF.Sigmoid)
                    nc.vector.tensor_tensor(sg[:], sg[:], pgs[i][:],
                                            _ALU.mult)
                    nc.vector.tensor_tensor(dst[:, fb, o:o + sz], sg[:],
                                            pus[i][:], _ALU.mult)

    def down_phase(a_sb, d_sb, ntb, out_d, scale, split_out=False):
        """Down-projection, activation-stationary (lhs = a[f, tok-block]),
        streaming the [f, H] weights as the moving operand."""
        for tb in range(ntb):
            tbs = slice(tb * P, (tb + 1) * P)
            pds = [mmp.tile([P, 512], _F32, tag=f"p{hc}", bufs=2,
                            name=f"pd{hc}") for hc in range(HC)]
            for fb in range(FBN):
                for hc in range(HC):
                    nc.tensor.matmul(
                        pds[hc][:], a_sb[:, fb, tbs],
                        d_sb[:, fb, hc * 512:(hc + 1) * 512],
                        start=(fb == 0), stop=(fb == FBN - 1),
                    )
            o = opool.tile([P, H], _F32, tag="o", bufs=3, name="o")
            for hc in range(HC):
                hs = slice(hc * 512, (hc + 1) * 512)
                if scale:
                    nc.vector.tensor_scalar_mul(o[:, hs], pds[hc][:],
                                                wr_sb[:, tb:tb + 1])
                else:
                    nc.scalar.copy(o[:, hs], pds[hc][:])
                if split_out:
                    eng = nc.scalar if hc % 2 == 0 else nc.sync
                    eng.dma_start(out_d[tbs, hs], o[:, hs])
            if not split_out:
                eng = nc.scalar if tb % 2 == 0 else nc.sync
                eng.dma_start(out_d[tbs, :], o[:])

    r_chunks = [(0, 512), (512, C - 512)]
    s_chunks = [(0, 512)]
    gu_phase(wg_d, wu_d, xg_sb, r_chunks, aT)
    down_phase(aT, dw_sb, CB, outr_d, scale=True)
    # sd reuses dw's SBUF slot: its DMA waits for the routed down phase's
    # last dw read, landing well before the shared down phase needs it
    sd_sb = dpool.tile([P, FBN, H], _BF16, tag="dwn", name="sd_sb")
    nc.scalar.dma_start(sd_sb[:], sd_d[:])
    gu_phase(sg_d, su_d, xs_sb, s_chunks, asT)
    down_phase(asT, sd_sb, SB, outs_d, scale=False, split_out=True)


def build_program():
    nc = bacc.Bacc("TRN2", target_bir_lowering=False, debug=False)
    with tile.TileContext(nc) as tc:
        with ExitStack() as ctx:
            _build_body(ctx, tc)
    nc.compile()
    return nc


def _get_nc():
    global _CACHED_NC
    if _CACHED_NC is None:
        _CACHED_NC = build_program()
    return _CACHED_NC


def _route(h, gate_weight):
    """Mirror of the reference's softmax + group-limited top-2, numpy fp32."""
    logits = (h @ gate_weight.T).astype(F32)
    m = logits.max(-1, keepdims=True)
    e = np.exp(logits - m)
    scores = e / e.sum(-1, keepdims=True)                     # [T, N]
    E = NEXP // N_GROUP
    gs = scores.reshape(T, N_GROUP, E).max(-1)                # [T, G]
    gidx = np.argsort(-gs, axis=1, kind="stable")[:, :TOPK_GROUP]
    gmask = np.zeros((T, N_GROUP), F32)
    np.put_along_axis(gmask, gidx, 1.0, axis=1)
    masked = np.where(np.repeat(gmask, E, axis=1) > 0, scores, 0.0)
    ti = np.argsort(-masked, axis=1, kind="stable")[:, :TOP_K]  # [T, K]
    tw = np.take_along_axis(masked, ti, axis=1)               # [T, K]
    return tw, ti


def _wslab(w, half=None):
    """[F', H] row-major weight -> [P, FBN, KO, P] bf16 lhsT layout."""
    m = w if half is None else w[half * F:(half + 1) * F]
    return np.ascontiguousarray(
        m.reshape(FBN, P, KO, P).transpose(3, 0, 2, 1).astype(BF16))


def _dslab(w, half=None):
    """[H, F'] down weight -> [P, FBN, H] bf16 (f-inner, fb, h') layout."""
    m = w if half is None else w[:, half * F:(half + 1) * F]
    return np.ascontiguousarray(
        m.T.reshape(FBN, P, H).transpose(1, 0, 2).astype(BF16))


def _xslab(hT, pad_to):
    """[H, t] f32 column-slice of tokens -> [P, KO, pad_to] bf16."""
    t = hT.shape[1]
    out = np.zeros((P, KO, pad_to), BF16)
    out[:, :, :t] = hT.reshape(KO, P, t).transpose(1, 0, 2).astype(BF16)
    return out


def prepare(inputs):
    h = np.asarray(inputs["hidden_states"], F32).reshape(T, H)
    hT = np.ascontiguousarray(h.T)                            # [H, T]
    tw, ti = _route(h, np.asarray(inputs["gate_weight"], F32))

    gate_w = np.asarray(inputs["gate_w"], F32)
    up_w = np.asarray(inputs["up_w"], F32)
    down_w = np.asarray(inputs["down_w"], F32)

    # shared-expert shards, built once and referenced by multiple cores
    sgh = [_wslab(np.asarray(inputs["sh_gate_w"], F32), hn) for hn in (0, 1)]
    suh = [_wslab(np.asarray(inputs["sh_up_w"], F32), hn) for hn in (0, 1)]
    sdh = [_dslab(np.asarray(inputs["sh_down_w"], F32), hn) for hn in (0, 1)]
    xsq = [_xslab(hT[:, q * TS:(q + 1) * TS], TS) for q in range(4)]

    in_maps, gathers, overflows = [], [], []
    for n in range(NEXP):
        sel = np.nonzero(ti == n)
        idx = sel[0]                                          # token ids
        w = tw[sel[0], sel[1]]                                # combine weights
        if len(idx) > C:
            overflows.append((n, idx[C:], w[C:]))
            idx, w = idx[:C], w[:C]
        gathers.append((idx, len(idx)))
        wr = np.zeros(C, F32)
        wr[:len(idx)] = w
        in_maps.append({
            "xg": _xslab(hT[:, idx], C),
            "xs": xsq[n // 2],
            "wg": _wslab(gate_w[n]),
            "wu": _wslab(up_w[n]),
            "sg": sgh[n % 2],
            "su": suh[n % 2],
            "dw": _dslab(down_w[n]),
            "sd": sdh[n % 2],
            "wr": np.ascontiguousarray(wr.reshape(CB, P).T),
        })
    return in_maps, gathers, overflows


def _silu(x):
    return x / (1.0 + np.exp(-x))


def run(inputs, trace=False, **kwargs):
    nc = _get_nc()
    in_maps, gathers, overflows = prepare(inputs)
    res = run_bass_kernel_spmd(
        nc, in_maps, core_ids=list(range(NEXP)), trace=trace, **kwargs
    )
    out = np.empty((T, H), F32)
    for q in range(4):
        out[q * TS:(q + 1) * TS] = (res.results[2 * q]["outs"]
                                    + res.results[2 * q + 1]["outs"])
    for n in range(NEXP):
        idx, cnt = gathers[n]
        out[idx] += res.results[n]["outr"][:cnt]
    for n, idx, w in overflows:   # exact host fallback, normally empty
        x = np.asarray(inputs["hidden_states"], F32).reshape(T, H)[idx]
        g = x @ np.asarray(inputs["gate_w"][n], F32).T
        u = x @ np.asarray(inputs["up_w"][n], F32).T
        out[idx] += ((_silu(g) * u) @ np.asarray(inputs["down_w"][n], F32).T
                     * w[:, None])
    return out.reshape(B, S, H), res


def kernel(**inputs):
    out, _ = run(inputs)
    return out
